# revision 1
# baseline (speedup 1.0000x reference)
"""Trainium2 Bass kernel for ActivationHyperbolic (Poincare ball, relu activation).

Math (per row of x [N, 64], c scalar, s = sqrt(c)):
    xn  = max(||x||, 1e-7)
    arg = min(s*xn, 1 - 1e-7)
    u   = x * atanh(arg)/(s*xn)          (logmap0 + relu; relu commutes with
    u   = relu(u) = scale1 * relu(x)      the positive per-row scale)
    un  = ||u|| = scale1 * ||relu(x)||
    y   = u * tanh(s*un)/(s*un)
    out = y * min(maxnorm/||y||, 1),  maxnorm = (1-4e-3)/s

Everything collapses to out = relu(x) * total with per-row
    total = min(maxnorm / rn, scale1 * scale2)
    scale1 = atanh(arg)/(s*xn),  scale2 = tanh(s*un)/(s*un),  rn = ||relu(x)||

Only two row-reductions are needed: A = sum(x^2), B = sum(relu(x)^2).
All transcendentals are built from Ln/Exp (one ACT table set):
    sqrt(v) = exp(0.5*ln(v)),  atanh(a) = 0.5*(ln(1+a) - ln(1-a)),
    tanh(z)/z = (e^{2z}-1) / (z*(e^{2z}+1))

Sharding: pure data-parallel, rows split evenly across 8 NeuronCores.
"""

import math
import sys

import numpy as np

for _p in ("/opt/trn_rl_repo",):
    if _p not in sys.path:
        sys.path.insert(0, _p)

import concourse.bass as bass
import concourse.tile as tile
from concourse import mybir
from concourse.bass_utils import run_bass_kernel_spmd

P = 128                      # SBUF partitions
D = 64                       # feature dim
NCORES = 8
N_TOTAL = 2097152
ROWS = N_TOTAL // NCORES     # 262144 rows per core
K = 32                       # row-groups per x-tile (free dim = K*D = 2048)
TILE_ROWS = P * K            # 4096 rows per tile
NTILES = ROWS // TILE_ROWS   # 64 tiles per core
F = K * D                    # flat free dim per tile (2048)
G = 16                       # tiles per chain group (stats batch)

BALL_EPS = 4e-3
ATANH_EPS = 1e-7

AF = mybir.ActivationFunctionType
ALU = mybir.AluOpType
AX = mybir.AxisListType
F32 = mybir.dt.float32
BF16 = mybir.dt.bfloat16


def _split_dma_waits(nc: bass.Bass) -> None:
    """Walrus can encode only ONE semaphore wait on a PSEUDO_DMA_DIRECT2D
    instruction (NEURON_ISA_TPB_EVENTS has a single wait slot). Tile may
    attach 2-3 waits to a DMA (slot-reuse WAR + queue WAW). Hoist all but
    one wait onto standalone event-semaphore instructions executed by the
    same engine immediately before the DMA — same semantics, encodable."""
    for f in nc.m.functions:
        for bb in f.blocks:
            new_insts = []
            for ins in bb.instructions:
                si = ins.sync_info
                if (
                    si is not None
                    and si.on_wait
                    and len(si.on_wait) > 1
                    and not isinstance(ins, mybir.InstEventSemaphore)
                ):
                    waits = list(si.on_wait)
                    for wsub in waits[:-1]:
                        wi = mybir.InstEventSemaphore(
                            name=f"I-dmawait-{nc.next_id()}",
                            ins=[],
                            outs=[],
                            engine=ins.engine,
                        )
                        wi.sync_info = mybir.SyncInfo(
                            on_wait=[wsub], on_update=[]
                        )
                        new_insts.append(wi)
                    ins.sync_info = mybir.SyncInfo(
                        on_wait=[waits[-1]], on_update=list(si.on_update)
                    )
                new_insts.append(ins)
            bb.instructions[:] = new_insts


def _build(c_val: float) -> bass.Bass:
    s = math.sqrt(c_val)
    ln_s = math.log(s)
    inv_s = 1.0 / s
    maxnorm = (1.0 - BALL_EPS) / s
    m = 1.0 - ATANH_EPS

    nc = bass.Bass()

    # Register the activation bias constants (bias floats are auto-converted
    # to [128,1] const APs; only 0.0/1.0 are pre-registered by Bass).
    def _register_const(value: float):
        if (F32, value) in nc.const_aps.aps:
            return
        t = nc.alloc_sbuf_tensor(f"const-f32-{value}", [128, 1], F32)
        nc.gpsimd.memset(t.ap(), value)
        nc.const_aps.aps[(F32, value)] = t.ap()

    for v in (ln_s, -ln_s, m, 1.0 + m, 1.0 - m, -1e-20, 1e-20):
        _register_const(float(v))
    nc.all_engine_barrier()

    x = nc.declare_dram_parameter("x", [ROWS, D], F32, isOutput=False)
    out = nc.declare_dram_parameter("out", [ROWS, D], F32, isOutput=True)
    xr = x[:].rearrange("(t p k) d -> t p (k d)", p=P, k=K)
    outr = out[:].rearrange("(t p k) d -> t p (k d)", p=P, k=K)

    with tile.TileContext(nc, pool_alloc_mode="queue") as tc:
        with (
            tc.tile_pool(name="xin", bufs=3) as xin_pool,
            tc.tile_pool(name="rpool", bufs=G + 4) as r_pool,
            tc.tile_pool(name="sq", bufs=2) as sq_pool,
            tc.tile_pool(name="opool", bufs=4) as o_pool,
            tc.tile_pool(name="stats", bufs=2) as stats_pool,
            tc.tile_pool(name="chain", bufs=1) as chain_pool,
        ):
            def stream_group(tiles):
                cf = len(tiles) * K
                A = stats_pool.tile([P, cf], F32, tag="A", name="A")
                B = stats_pool.tile([P, cf], F32, tag="B", name="B")
                rs = []
                for j, t in enumerate(tiles):
                    xt = xin_pool.tile([P, F], F32, tag="x", name="x")
                    nc.sync.dma_start(out=xt[:], in_=xr[t])
                    xsq = sq_pool.tile([P, F], F32, tag="xsq", name="xsq")
                    nc.scalar.activation(xsq[:], xt[:], AF.Square)
                    nc.vector.reduce_sum(
                        A[:, j * K : (j + 1) * K],
                        xsq[:].rearrange("p (k d) -> p k d", d=D),
                        axis=AX.X,
                    )
                    # r = relu(x) in bf16 (halves resident-pool bytes)
                    r = r_pool.tile([P, F], BF16, tag="r", name="r")
                    nc.scalar.activation(r[:], xt[:], AF.Relu)
                    rsq = sq_pool.tile([P, F], BF16, tag="rsq", name="rsq")
                    # rsq = r*r split DVE/ACT; POOL stays out of the
                    # mid-pipeline (its serial FIFO would delay reductions)
                    if t % 4 == 0:
                        nc.vector.tensor_tensor(rsq[:], r[:], r[:], ALU.mult)
                    elif t % 4 == 1:
                        nc.gpsimd.tensor_tensor(rsq[:], r[:], r[:], ALU.mult)
                    else:
                        nc.scalar.activation(rsq[:], r[:], AF.Square)
                    nc.vector.reduce_sum(
                        B[:, j * K : (j + 1) * K],
                        rsq[:].rearrange("p (k d) -> p k d", d=D),
                        axis=AX.X,
                    )
                    rs.append((t, r))
                return A, B, rs

            def tail_group(A, B, rs, endgame=False):
                cf = len(rs) * K

                def ct(tag):
                    return chain_pool.tile([P, cf], F32, tag=tag, name=tag)

                # Per-row chain on [P, cf]; 6 reusable tiles (q1..q6).
                q1 = ct("q1"); q2 = ct("q2"); q3 = ct("q3")
                q4 = ct("q4"); q5 = ct("q5"); q6 = ct("q6")
                # A-path
                nc.scalar.activation(q1[:], A[:], AF.Ln)                 # L
                nc.scalar.activation(q2[:], q1[:], AF.Exp, scale=0.5, bias=ln_s)   # argu
                nc.scalar.activation(q2[:], q2[:], AF.Relu, scale=-1.0, bias=m)    # w
                nc.scalar.activation(q3[:], q2[:], AF.Ln, scale=-1.0, bias=1.0 + m)  # ln(1+arg)
                nc.scalar.activation(q2[:], q2[:], AF.Ln, scale=1.0, bias=1.0 - m)   # ln(1-arg)
                nc.scalar.activation(q4[:], q1[:], AF.Exp, scale=-0.5, bias=-ln_s)   # 1/(s xn)
                # B-path ACT ops (independent of the DVE block below)
                nc.scalar.activation(q1[:], B[:], AF.Relu, bias=-1e-20)  # max(B,..)
                nc.scalar.activation(q1[:], q1[:], AF.Ln, bias=1e-20)    # M
                nc.scalar.activation(q5[:], q1[:], AF.Exp, scale=0.5)    # rn
                nc.scalar.activation(q6[:], q1[:], AF.Exp, scale=-0.5)   # 1/rn
                nc.vector.tensor_sub(q3[:], q3[:], q2[:])                # 2 atanh
                nc.vector.scalar_tensor_tensor(
                    q3[:], q3[:], 0.5, q4[:], ALU.mult, ALU.mult
                )                                                        # scale1
                nc.vector.tensor_mul(q5[:], q3[:], q5[:])                # un
                nc.scalar.activation(q1[:], q5[:], AF.Exp, scale=2.0 * s)  # E
                nc.vector.scalar_tensor_tensor(
                    q2[:], q1[:], 1.0, q5[:], ALU.add, ALU.mult
                )                                                        # (E+1) un
                q4b = ct("q4b")
                nc.vector.reciprocal(q4b[:], q2[:])
                nc.vector.tensor_scalar_add(q1[:], q1[:], -1.0)          # E-1
                nc.vector.scalar_tensor_tensor(
                    q1[:], q1[:], inv_s, q4b[:], ALU.mult, ALU.mult
                )                                                        # scale2
                nc.vector.tensor_mul(q3[:], q3[:], q1[:])                # s12
                total = ct("total")
                nc.vector.scalar_tensor_tensor(
                    total[:], q6[:], maxnorm, q3[:], ALU.mult, ALU.min
                )                                                        # total
                total_bf = chain_pool.tile([P, cf], BF16, tag="tbf", name="tbf")
                nc.vector.tensor_copy(total_bf[:], total[:])

                for j, (t, r) in enumerate(rs):
                    tb = total_bf[:, j * K : (j + 1) * K].to_broadcast((P, K, D))
                    ot = o_pool.tile([P, F], F32, tag="o", name="o")
                    o3 = ot[:].rearrange("p (k d) -> p k d", d=D)
                    r3 = r[:].rearrange("p (k d) -> p k d", d=D)
                    # out = relu(x) * total (POOL; split with DVE in the
                    # endgame where other engines are idle)
                    if endgame and j % 2 == 1:
                        nc.vector.tensor_tensor(o3, r3, tb, ALU.mult)
                    else:
                        nc.gpsimd.tensor_tensor(o3, r3, tb, ALU.mult)
                    nc.sync.dma_start(out=outr[t], in_=ot[:])

            # Software pipeline: emit group sg's streaming ops, THEN the
            # previous group's chain+finals — so the serial chain's waits
            # are pre-satisfied when each engine reaches them instead of
            # head-of-line blocking the next group's streaming work.
            group_sizes = [16, 16, 16, 16]
            assert sum(group_sizes) == NTILES
            starts = [sum(group_sizes[:i]) for i in range(len(group_sizes))]
            pending = None
            for gs, st in zip(group_sizes, starts):
                cur = stream_group(list(range(st, st + gs)))
                if pending is not None:
                    tail_group(*pending)
                pending = cur
            tail_group(*pending, endgame=True)

    _split_dma_waits(nc)
    return nc


_BUILD_CACHE: dict[float, bass.Bass] = {}


def _run(x: np.ndarray, c: np.ndarray, trace: bool = False):
    x = np.ascontiguousarray(x, dtype=np.float32)
    assert x.shape == (N_TOTAL, D), x.shape
    c_val = float(np.asarray(c).reshape(-1)[0])
    nc = _BUILD_CACHE.get(c_val)
    if nc is None:
        nc = _build(c_val)
        _BUILD_CACHE[c_val] = nc
    shards = np.split(x, NCORES, axis=0)
    in_maps = [{"x": sh} for sh in shards]
    res = run_bass_kernel_spmd(
        nc, in_maps, core_ids=list(range(NCORES)), trace=trace
    )
    out = np.concatenate([res.results[i]["out"] for i in range(NCORES)], axis=0)
    return out, res


def kernel(x: np.ndarray, c: np.ndarray) -> np.ndarray:
    out, _ = _run(x, c, trace=False)
    return out

